# revision 47
# baseline (speedup 1.0000x reference)
"""Trainium2 Bass kernel for a 2-layer ResGatedGraphConv encoder.

Strategy (edge-parallel over 8 NeuronCores):
  - Nodes are permuted by degree rank and dealt round-robin to the 8 cores, so
    each core owns NPC nodes arranged in TPC tiles of 128 dst nodes whose
    degrees are nearly uniform within a tile.
  - Each edge lives on the core/tile/partition of its dst node; per tile the
    edge list of every dst node is padded to the tile max degree d_hat, so the
    per-edge gather lands token-major [128 dst, d_hat, 128] and aggregation is
    a plain free-axis reduction (no scatter, no indicator matmuls).
  - Per layer, a packed [q|v] node table [NT, 128] bf16 lives in DRAM;
    messages gather rows of it with one indirect DMA per (tile, slot).
  - Each core builds only the table rows of ITS OWN nodes (layer 1 from the
    SBUF-resident x shard; layer 2 fused into the conv-1 epilogue from the
    SBUF-resident h1), then one AllGather per layer replicates the table.
    x and h1 stay in SBUF for the whole program: no per-tile hot loads and
    no h1 round-trip through DRAM.

kernel(**inputs) takes the full (unsharded) inputs and returns the full
output; all sharding happens inside.
"""

import sys
import numpy as np

for _p in ("/opt/trn_rl_repo", "/opt/pypackages"):
    if _p not in sys.path:
        sys.path.append(_p)

N = 100000
E = 1600000
H = 64
NCORES = 8


class Cfg:
    def __init__(self, n, tpc, nch=7):
        self.n = n
        self.tpc = tpc                      # dst tiles per core
        self.npc = tpc * 128                # nodes per core
        self.npad = NCORES * self.npc       # padded node count
        self.nt = self.npad + 128           # table rows (incl. dummy block)
        self.dummy = self.npad              # dummy (all-zero) table row
        self.nch = nch                      # AllGather chunks
        self.sch = self.npc // nch          # slots per chunk (per core)
        assert self.sch * nch == self.npc and self.sch % 128 == 0
        assert self.npad >= n


FULL_CFG = Cfg(N, 98)


def host_prep(x, edge_index, cfg):
    """Permute nodes / build per-core gather schedules on the host."""
    n = cfg.n
    src = np.asarray(edge_index[0]).astype(np.int64)
    dst = np.asarray(edge_index[1]).astype(np.int64)
    deg = np.bincount(dst, minlength=n)

    # degree-rank round-robin: rank r -> core r%8, slot r//8
    rank_order = np.argsort(deg, kind="stable")  # node ids in degree order
    r = np.arange(n)
    node_core = np.empty(n, np.int64)
    node_slot = np.empty(n, np.int64)
    # slots 0..(pads-1) of every core are left empty (pad nodes); their
    # table rows live in AllGather chunk 0 and are zeroed on device, so
    # row 0 serves as the all-zero dummy gather target.
    pads = cfg.npc - n // NCORES
    node_core[rank_order] = r % NCORES
    node_slot[rank_order] = pads + r // NCORES
    tau = node_core * cfg.npc + node_slot      # (core, slot) id of each node
    # chunk-major table row: AllGather chunk k lands at contiguous rows
    # [8*sch*k, 8*sch*(k+1))
    tau_tab = ((node_slot // cfg.sch) * (NCORES * cfg.sch)
               + node_core * cfg.sch + node_slot % cfg.sch)

    # per-edge position within its dst node's list, sorted ascending by
    # src table row so low slots only need early AllGather chunks
    order = np.lexsort((tau_tab[src], dst))
    d_sorted = dst[order]
    first = np.searchsorted(d_sorted, np.arange(n))
    k_within = np.arange(len(dst)) - first[d_sorted]
    k_e = np.empty(len(dst), np.int64)
    k_e[order] = k_within

    e_core = node_core[dst]
    e_slot = node_slot[dst]
    e_tile = e_slot // 128
    e_part = e_slot % 128

    # per-tile max degree (shared across cores so programs are identical)
    deg_cs = np.zeros((NCORES, cfg.npc), np.int64)
    deg_cs[node_core, node_slot] = deg
    d_hat = deg_cs.reshape(NCORES, cfg.tpc, 128).max(axis=(0, 2))
    d_hat = np.maximum(d_hat, 1).astype(np.int64)
    off = np.concatenate([[0], np.cumsum(d_hat)])
    nblk = int(off[-1])

    gidx = np.zeros((NCORES, 128, nblk), np.int32)   # pads -> zero row 0
    col = off[e_tile] + k_e
    gidx[e_core, e_part, col] = tau_tab[src].astype(np.int32)

    # classify gather columns by the last AllGather chunk they need, pick
    # up to EMAX "early" columns (gatherable while conv1 still runs), and
    # reorder each tile's columns early-first.
    EMAX = 512
    sch8 = NCORES * cfg.sch
    tch = cfg.sch // 128
    need_k = gidx.max(axis=(0, 1)) // sch8           # [nblk], shared
    elig = np.where(need_k <= cfg.nch - 2)[0]
    sel = set(elig[np.argsort(need_k[elig], kind="stable")][:EMAX].tolist())

    # reorder each tile's columns early-first; assign etab2 storage ids
    new_perm = []
    ne_t, e0_t, early_items = [], [], []
    e_ctr = 0
    for t in range(cfg.tpc):
        cols = list(range(off[t], off[t + 1]))
        ear = [c for c in cols if c in sel]
        late = [c for c in cols if c not in sel]
        new_perm += ear + late
        e0_t.append(e_ctr)
        ne_t.append(len(ear))
        for i, c in enumerate(ear):
            early_items.append((e_ctr, off[t] + i, int(need_k[c])))
            e_ctr += 1
    gidx = gidx[:, :, new_perm]

    # per-conv1-tile issue schedule for the early gathers
    issue = sorted(early_items, key=lambda it: (it[2], it[0]))
    sched = [[] for _ in range(cfg.tpc)]
    qi = 0
    for t1 in range(cfg.tpc):
        kav = (t1 - 3) // tch - 1
        while (qi < len(issue) and len(sched[t1]) < 8
               and issue[qi][2] <= kav):
            sched[t1].append(issue[qi])
            qi += 1
    while qi < len(issue):
        sched[cfg.tpc - 1].append(issue[qi])
        qi += 1

    # per-core x shard (feature-major, slot order) and edge src-id map
    import ml_dtypes
    xT = np.asarray(x, np.float32).T.astype(ml_dtypes.bfloat16)
    xT_own = []
    for c in range(NCORES):
        sh = np.zeros((H, cfg.npc), ml_dtypes.bfloat16)
        nodes = np.where(node_core == c)[0]
        sh[:, node_slot[nodes]] = xT[:, nodes]
        xT_own.append(sh)

    # per-core edge-expanded features for the streamed layer-1 conv:
    # xET[c][:, (col*128 + p)] = x[src of edge at (col, p)] (pads -> 0)
    sid = np.full((NCORES, 128, nblk), -1, np.int64)
    sid[e_core, e_part, col] = src
    sid = sid[:, :, new_perm]
    xT_pad = np.concatenate(
        [xT, np.zeros((H, 1), ml_dtypes.bfloat16)], axis=1)
    xET = [np.ascontiguousarray(xT_pad[:, sid[c].T.ravel()])
           for c in range(NCORES)]

    return dict(
        gidx=gidx,
        d_hat=[int(v) for v in d_hat],
        nblk=nblk,
        tau=tau,
        xT_own=xT_own,
        xET=xET,
        ne_t=ne_t,
        e0_t=e0_t,
        n_early=e_ctr,
        sched=sched,
        pads=pads,
    )


def build_program(cfg, d_hat, nblk, ne_t, e0_t, n_early, sched, pads):
    import concourse.bass as bass
    import concourse.bacc as bacc
    import concourse.mybir as mybir
    import concourse.tile as tile
    from concourse.masks import make_identity

    f32 = mybir.dt.float32
    bf16 = mybir.dt.bfloat16
    tpc, npc, nt = cfg.tpc, cfg.npc, cfg.nt

    nc = bacc.Bacc("TRN2", target_bir_lowering=False, debug=False,
                   num_devices=NCORES)

    # ---- I/O (weights/x are host-cast to bf16) ----
    xT_own = nc.dram_tensor("xT_own", [H, npc], bf16, kind="ExternalInput")
    xET = nc.dram_tensor("xET", [H, nblk * 128], bf16, kind="ExternalInput")
    gidx = nc.dram_tensor("gidx", [128, nblk], mybir.dt.int32,
                          kind="ExternalInput")
    wnames = {}
    for l in (1, 2):
        for w in ("Wqv", "Wk", "Ws", "Wl"):
            shape = [H, 128] if w == "Wqv" else [H, H]
            wnames[f"{w}{l}"] = nc.dram_tensor(f"{w}{l}", shape, bf16,
                                               kind="ExternalInput")
        wnames[f"b{l}r"] = nc.dram_tensor(f"b{l}r", [1, H], bf16,
                                          kind="ExternalInput")
        wnames[f"bl{l}c"] = nc.dram_tensor(f"bl{l}c", [H, 1], f32,
                                           kind="ExternalInput")
    out_shard = nc.dram_tensor("out_shard", [npc, H], f32,
                               kind="ExternalOutput")

    # ---- internal DRAM (layer-2 q|v table only; layer 1 is streamed) ----
    qv2_t = nc.dram_tensor("qv2_t", [nt, 128], bf16, addr_space="Shared")
    qv2_sh = nc.dram_tensor("qv2_sh", [npc, 128], bf16)
    etab2 = nc.dram_tensor("etab2", [max(1, n_early) * 128, 128], bf16)

    with tile.TileContext(nc) as tc:
        cp = tc.alloc_tile_pool(name="const", bufs=1)

        identity = cp.tile([128, 128], f32)
        make_identity(nc, identity[:])
        identity64 = cp.tile([64, 64], f32)
        make_identity(nc, identity64[:])
        ones_row = cp.tile([1, 128], bf16)
        nc.vector.memset(ones_row[:], 1.0)
        zeros128 = cp.tile([128, 128], bf16)
        nc.vector.memset(zeros128[:], 0.0)

        gidx_sb = cp.tile([128, nblk], mybir.dt.int32)
        nc.sync.dma_start(out=gidx_sb[:], in_=gidx.ap()[:])

        # x shard and h1 live in SBUF for the whole program
        xsb = cp.tile([H, npc], bf16)
        nc.sync.dma_start(out=xsb[:], in_=xT_own.ap()[:])
        h1_all = cp.tile([H, npc], bf16)

        wt = {}
        for l in (1, 2):
            for w in ("Wqv", "Wk", "Ws", "Wl"):
                shape = [H, 128] if w == "Wqv" else [H, H]
                wt[f"{w}{l}"] = cp.tile(shape, bf16, name=f"{w}{l}",
                                        tag=f"{w}{l}")
                nc.sync.dma_start(out=wt[f"{w}{l}"][:],
                                  in_=wnames[f"{w}{l}"].ap()[:])
            wt[f"b{l}r"] = cp.tile([1, H], bf16, name=f"b{l}r", tag=f"b{l}r")
            nc.sync.dma_start(out=wt[f"b{l}r"][:], in_=wnames[f"b{l}r"].ap()[:])
            wt[f"bl{l}c"] = cp.tile([H, 1], f32, name=f"bl{l}c", tag=f"bl{l}c")
            nc.sync.dma_start(out=wt[f"bl{l}c"][:],
                              in_=wnames[f"bl{l}c"].ap()[:])

        def table_exchange_chunk(k):
            """AllGather shard chunk k into contiguous table rows."""
            r0, r1 = k * cfg.sch, (k + 1) * cfg.sch
            nc.gpsimd.collective_compute(
                "AllGather",
                mybir.AluOpType.bypass,
                replica_groups=[list(range(NCORES))],
                ins=[qv2_sh.ap()[r0:r1, :]],
                outs=[qv2_t.ap()[NCORES * r0:NCORES * r1, :]],
            )

        # -------- layer 1: streamed conv (no gather; token-major) ----------
        def conv1_streamed():
            G = 4   # qv chunks per PSUM group
            TCH = cfg.sch // 128  # tiles per AllGather chunk
            wqv, wk, ws, wl = (wt["Wqv1"], wt["Wk1"], wt["Ws1"], wt["Wl1"])
            brow, blc = wt["b1r"], wt["bl1c"]
            with tc.tile_pool(name="p1e", bufs=2) as pe_, \
                 tc.tile_pool(name="p1s", bufs=3) as pbs, \
                 tc.tile_pool(name="p1p", bufs=2, space="PSUM") as pbp:
                off_c = 0
                for t in range(tpc):
                    dh = d_hat[t]
                    hot = xsb[:, t * 128:(t + 1) * 128]

                    # per-edge x features of this tile, streamed in
                    xe = pe_.tile([H, dh * 128], bf16, tag="xe", bufs=6)
                    nc.sync.dma_start(
                        out=xe[:],
                        in_=xET.ap()[:, off_c * 128:(off_c + dh) * 128])

                    # k tile for this dst tile: [128 n, 64 h]
                    kps = pbp.tile([128, H], f32, tag="kps", space="PSUM",
                                   bufs=1)
                    nc.tensor.matmul(kps[:], lhsT=hot, rhs=wk[:],
                                     start=True, stop=True)
                    ksb = pbs.tile([128, H], bf16, tag="ksb", bufs=2)
                    nc.scalar.activation(ksb[:], kps[:],
                                         mybir.ActivationFunctionType.Copy)

                    msg = pbs.tile([128, dh, H], bf16, tag="msg", bufs=4)
                    for j0 in range(0, dh, G):
                        gc = min(G, dh - j0)
                        # expand gc chunks of 128 edges: [128 e, 128 qv] each
                        psg = pbp.tile([128, G * 128], f32, tag="ps",
                                       space="PSUM", bufs=3)
                        for i in range(gc):
                            nc.tensor.matmul(
                                psg[:, i * 128:(i + 1) * 128],
                                lhsT=xe[:, (j0 + i) * 128:(j0 + i + 1) * 128],
                                rhs=wqv[:], start=True, stop=True)
                        psv = psg[:, 0:gc * 128].rearrange(
                            "p (g e) -> p g e", e=128)
                        # sigarg = q + k[dst]; sigmoid; msg = sig * v
                        sa = pbs.tile([128, G, H], bf16, tag="sa", bufs=4)
                        kb = ksb[:].rearrange("p (o h) -> p o h", o=1)
                        kb = bass.AP(kb.tensor, kb.offset,
                                     [kb.ap[0], [0, gc], kb.ap[2]])
                        nc.vector.tensor_tensor(
                            out=sa[:, 0:gc, :], in0=psv[:, :, 0:H], in1=kb,
                            op=mybir.AluOpType.add)
                        sg = pbs.tile([128, G, H], bf16, tag="sg", bufs=4)
                        nc.scalar.activation(
                            sg[:, 0:gc, :], sa[:, 0:gc, :],
                            mybir.ActivationFunctionType.Sigmoid)
                        nc.vector.tensor_tensor(
                            out=msg[:, j0:j0 + gc, :], in0=sg[:, 0:gc, :],
                            in1=psv[:, :, H:128],
                            op=mybir.AluOpType.mult)

                    # agg[p, h] = sum_j msg[p, j, h]
                    agg = pbs.tile([128, H], f32, tag="agg")
                    mt = msg[:].rearrange("p k h -> p h k")
                    nc.vector.tensor_reduce(
                        out=agg[:], in_=mt, axis=mybir.AxisListType.X,
                        op=mybir.AluOpType.add)

                    # conv out = agg + x@Ws + b  (token-major [128 n, 64 h])
                    cps = pbp.tile([128, H], f32, tag="cps", space="PSUM",
                                   bufs=2)
                    nc.tensor.matmul(cps[:], lhsT=hot, rhs=ws[:],
                                     start=True, stop=False)
                    nc.tensor.matmul(cps[:], lhsT=ones_row[:], rhs=brow[:],
                                     start=False, stop=True)
                    hc = pbs.tile([128, H], f32, tag="hc")
                    nc.vector.tensor_tensor(out=hc[:], in0=agg[:], in1=cps[:],
                                            op=mybir.AluOpType.add)

                    # transpose to feature-major, then linear + relu
                    tps = pbp.tile([H, 128], f32, tag="ps", space="PSUM",
                                   bufs=3)
                    nc.tensor.transpose(out=tps[:], in_=hc[:],
                                        identity=identity[:])
                    hcT = pbs.tile([H, 128], bf16, tag="hcT")
                    nc.scalar.activation(hcT[:], tps[:],
                                         mybir.ActivationFunctionType.Copy)
                    lps = pbp.tile([H, 128], f32, tag="lps", space="PSUM",
                                   bufs=2)
                    nc.tensor.matmul(lps[:], lhsT=wl[:], rhs=hcT[:],
                                     start=True, stop=True)

                    # h1 tile -> resident SBUF (feature-major, bf16)
                    h1t = h1_all[:, t * 128:(t + 1) * 128]
                    nc.scalar.activation(
                        h1t, lps[:],
                        mybir.ActivationFunctionType.Relu,
                        bias=blc[:])
                    # fused layer-2 table shard build (token-major rows)
                    qps = pbp.tile([128, 128], f32, tag="ps",
                                   space="PSUM", bufs=3)
                    nc.tensor.matmul(qps[:], lhsT=h1t, rhs=wt["Wqv2"][:],
                                     start=True, stop=True)
                    st2 = pbs.tile([128, 128], bf16, tag="st2")
                    nc.scalar.activation(st2[:], qps[:],
                                         mybir.ActivationFunctionType.Copy)
                    nc.sync.dma_start(
                        out=qv2_sh.ap()[t * 128:(t + 1) * 128, :],
                        in_=st2[:])
                    if t == 0:
                        # zero the pad-node table rows (incl. dummy row 0)
                        nc.sync.dma_start(out=qv2_sh.ap()[0:pads, :],
                                          in_=zeros128[0:pads, :])
                    if (t + 1) % TCH == 0:
                        table_exchange_chunk((t + 1) // TCH - 1)
                    # early layer-2 gathers: Q7 is otherwise idle in conv1
                    for (e, colid, needk) in sched[t]:
                        qe = pe_.tile([128, 128], bf16, tag="qe", bufs=8)
                        nc.gpsimd.indirect_dma_start(
                            out=qe[:], out_offset=None,
                            in_=qv2_t.ap()[
                                0:(needk + 1) * NCORES * cfg.sch, :],
                            in_offset=bass.IndirectOffsetOnAxis(
                                ap=gidx_sb[:, colid:colid + 1], axis=0),
                        )
                        nc.sync.dma_start(
                            out=etab2.ap()[e * 128:(e + 1) * 128, :],
                            in_=qe[:])
                    off_c += dh

        # -------- layer 2: gather-based conv (token-major) -----------------
        def conv_layer2():
            wk, ws, wl = wt["Wk2"], wt["Ws2"], wt["Wl2"]
            brow, blc = wt["b2r"], wt["bl2c"]
            with tc.tile_pool(name="pb2", bufs=2) as pb, \
                 tc.tile_pool(name="pbs2", bufs=3) as pbs, \
                 tc.tile_pool(name="pbp2", bufs=2, space="PSUM") as pbp:
                off_c = 0
                for t in range(tpc):
                    dh = d_hat[t]
                    hot = h1_all[:, t * 128:(t + 1) * 128]

                    # k tile for this dst tile: [128 n, 64 h]
                    kps = pbp.tile([128, H], f32, tag="kps", space="PSUM")
                    nc.tensor.matmul(kps[:], lhsT=hot, rhs=wk[:],
                                     start=True, stop=True)
                    ksb = pbs.tile([128, H], bf16, tag="ksb", bufs=4)
                    nc.scalar.activation(ksb[:], kps[:],
                                         mybir.ActivationFunctionType.Copy)

                    # q|v rows of this tile's edges: early columns were
                    # pre-gathered into etab2 during conv1; gather the rest
                    qvg_f = pb.tile([128, dh * 128], bf16, tag="qvg", bufs=8)
                    ne = ne_t[t]
                    if ne:
                        e0 = e0_t[t]
                        nc.sync.dma_start(
                            out=qvg_f[:, 0:ne * 128].rearrange(
                                "p (c e) -> p c e", e=128),
                            in_=etab2.ap()[e0 * 128:(e0 + ne) * 128,
                                           :].rearrange(
                                "(c p) e -> p c e", p=128))
                    for j in range(ne, dh):
                        nc.gpsimd.indirect_dma_start(
                            out=qvg_f[:, j * 128:(j + 1) * 128],
                            out_offset=None,
                            in_=qv2_t.ap()[:, :],
                            in_offset=bass.IndirectOffsetOnAxis(
                                ap=gidx_sb[:, off_c + j:off_c + j + 1],
                                axis=0),
                        )
                    qvg = qvg_f[:].rearrange("p (a b) -> p a b", b=128)

                    # sigarg = k[dst] + q ;  sig = sigmoid(sigarg)
                    sigarg = pbs.tile([128, dh, H], bf16, tag="sigarg", bufs=4)
                    kb = ksb[:].rearrange("p (o h) -> p o h", o=1)
                    kb = bass.AP(kb.tensor, kb.offset,
                                 [kb.ap[0], [0, dh], kb.ap[2]])
                    nc.vector.tensor_tensor(
                        out=sigarg[:], in0=qvg[:, :, 0:H], in1=kb,
                        op=mybir.AluOpType.add)
                    sig = pbs.tile([128, dh, H], bf16, tag="sig", bufs=4)
                    nc.scalar.activation(
                        sig[:], sigarg[:],
                        mybir.ActivationFunctionType.Sigmoid)
                    # msg = sig * v
                    msg = pbs.tile([128, dh, H], bf16, tag="msg", bufs=4)
                    nc.vector.tensor_tensor(
                        out=msg[:], in0=sig[:], in1=qvg[:, :, H:128],
                        op=mybir.AluOpType.mult)
                    # agg[p, h] = sum_k msg[p, k, h]
                    agg = pbs.tile([128, H], f32, tag="agg")
                    mt = msg[:].rearrange("p k h -> p h k")
                    nc.vector.tensor_reduce(
                        out=agg[:], in_=mt, axis=mybir.AxisListType.X,
                        op=mybir.AluOpType.add)

                    # conv out = agg + h1@Ws + b  (token-major [128 n, 64 h])
                    cps = pbp.tile([128, H], f32, tag="cps", space="PSUM")
                    nc.tensor.matmul(cps[:], lhsT=hot, rhs=ws[:],
                                     start=True, stop=False)
                    nc.tensor.matmul(cps[:], lhsT=ones_row[:], rhs=brow[:],
                                     start=False, stop=True)
                    hc = pbs.tile([128, H], f32, tag="hc")
                    nc.vector.tensor_tensor(out=hc[:], in0=agg[:], in1=cps[:],
                                            op=mybir.AluOpType.add)

                    # transpose to feature-major, then linear + relu
                    tps = pbp.tile([H, 128], f32, tag="tps", space="PSUM",
                                   bufs=1)
                    nc.tensor.transpose(out=tps[:], in_=hc[:],
                                        identity=identity[:])
                    hcT = pbs.tile([H, 128], bf16, tag="hcT")
                    nc.scalar.activation(hcT[:], tps[:],
                                         mybir.ActivationFunctionType.Copy)
                    lps = pbp.tile([H, 128], f32, tag="lps", space="PSUM")
                    nc.tensor.matmul(lps[:], lhsT=wl[:], rhs=hcT[:],
                                     start=True, stop=True)

                    h2T = pbs.tile([H, 128], f32, tag="h2T")
                    nc.scalar.activation(
                        h2T[:], lps[:],
                        mybir.ActivationFunctionType.Relu,
                        bias=blc[:])
                    ops = pbp.tile([128, H], f32, tag="kps", space="PSUM")
                    nc.tensor.transpose(out=ops[:], in_=h2T[:],
                                        identity=identity64[:])
                    osb = pbs.tile([128, H], f32, tag="osb")
                    nc.scalar.activation(osb[:], ops[:],
                                         mybir.ActivationFunctionType.Copy)
                    nc.sync.dma_start(
                        out=out_shard.ap()[t * 128:(t + 1) * 128, :],
                        in_=osb[:])
                    off_c += dh

        conv1_streamed()
        nc.sync.dma_start(out=qv2_t.ap()[cfg.npad:nt, :], in_=zeros128[:])
        tc.strict_bb_all_engine_barrier()
        conv_layer2()

        cp.release()

    nc.compile()
    return nc


def _pack_inputs(prep, inputs, cfg):
    """Build the 8 per-core input maps."""
    import ml_dtypes
    bf16 = ml_dtypes.bfloat16
    base = {}
    for l, (wq, wv, wk, ws, b, wl, bl) in {
        1: ("Wq1", "Wv1", "Wk1", "Ws1", "b1", "Wl1", "bl1"),
        2: ("Wq2", "Wv2", "Wk2", "Ws2", "b2", "Wl2", "bl2"),
    }.items():
        base[f"Wqv{l}"] = np.ascontiguousarray(
            np.concatenate([np.asarray(inputs[wq], np.float32),
                            np.asarray(inputs[wv], np.float32)],
                           axis=1)).astype(bf16)
        base[f"Wk{l}"] = np.asarray(inputs[wk], np.float32).astype(bf16)
        base[f"Ws{l}"] = np.asarray(inputs[ws], np.float32).astype(bf16)
        base[f"Wl{l}"] = np.asarray(inputs[wl], np.float32).astype(bf16)
        base[f"b{l}r"] = np.asarray(inputs[b], np.float32).reshape(1, H).astype(bf16)
        base[f"bl{l}c"] = np.ascontiguousarray(
            np.asarray(inputs[bl], np.float32).reshape(H, 1))

    in_maps = []
    for c in range(NCORES):
        m = dict(base)
        m["xT_own"] = prep["xT_own"][c]
        m["xET"] = prep["xET"][c]
        m["gidx"] = np.ascontiguousarray(prep["gidx"][c])
        in_maps.append(m)
    return in_maps


def run(inputs, cfg=FULL_CFG, sim=False, trace=False):
    from concourse import bass_utils

    x = np.asarray(inputs["x"], np.float32)
    prep = host_prep(x, inputs["edge_index"], cfg)
    nc = build_program(cfg, prep["d_hat"], prep["nblk"], prep["ne_t"],
                       prep["e0_t"], prep["n_early"], prep["sched"],
                       prep["pads"])
    in_maps = _pack_inputs(prep, inputs, cfg)

    if sim:
        from concourse.bass_interp import MultiCoreSim
        ms = MultiCoreSim(nc, num_cores=NCORES, trace=False)
        for c in range(NCORES):
            for name, arr in in_maps[c].items():
                ms.cores[c].tensor(name)[:] = arr
        ms.simulate(check_with_hw=False)
        shards = [np.array(ms.cores[c].tensor("out_shard")) for c in
                  range(NCORES)]
        res = None
    else:
        if trace:
            try:
                sys.path.insert(0, "/root/problem")
                import ntff_hook  # noqa: F401
            except Exception:
                trace = False
        res = bass_utils.run_bass_kernel_spmd(
            nc, in_maps, core_ids=list(range(NCORES)), trace=trace)
        shards = [res.results[c]["out_shard"] for c in range(NCORES)]

    full = np.concatenate(shards, axis=0)     # [npad, H] in permuted order
    out = np.empty((cfg.n, H), np.float32)
    out[:, :] = full[prep["tau"], :]
    return out, res


def kernel(**inputs):
    out, _ = run(inputs, FULL_CFG, sim=False, trace=False)
    return out.astype(np.float32)


# revision 48
# speedup vs baseline: 1.0066x; 1.0066x over previous
"""Trainium2 Bass kernel for a 2-layer ResGatedGraphConv encoder.

Strategy (edge-parallel over 8 NeuronCores):
  - Nodes are permuted by degree rank and dealt round-robin to the 8 cores, so
    each core owns NPC nodes arranged in TPC tiles of 128 dst nodes whose
    degrees are nearly uniform within a tile.
  - Each edge lives on the core/tile/partition of its dst node; per tile the
    edge list of every dst node is padded to the tile max degree d_hat, so the
    per-edge gather lands token-major [128 dst, d_hat, 128] and aggregation is
    a plain free-axis reduction (no scatter, no indicator matmuls).
  - Per layer, a packed [q|v] node table [NT, 128] bf16 lives in DRAM;
    messages gather rows of it with one indirect DMA per (tile, slot).
  - Each core builds only the table rows of ITS OWN nodes (layer 1 from the
    SBUF-resident x shard; layer 2 fused into the conv-1 epilogue from the
    SBUF-resident h1), then one AllGather per layer replicates the table.
    x and h1 stay in SBUF for the whole program: no per-tile hot loads and
    no h1 round-trip through DRAM.

kernel(**inputs) takes the full (unsharded) inputs and returns the full
output; all sharding happens inside.
"""

import sys
import numpy as np

for _p in ("/opt/trn_rl_repo", "/opt/pypackages"):
    if _p not in sys.path:
        sys.path.append(_p)

N = 100000
E = 1600000
H = 64
NCORES = 8


class Cfg:
    def __init__(self, n, tpc, nch=7):
        self.n = n
        self.tpc = tpc                      # dst tiles per core
        self.npc = tpc * 128                # nodes per core
        self.npad = NCORES * self.npc       # padded node count
        self.nt = self.npad + 128           # table rows (incl. dummy block)
        self.dummy = self.npad              # dummy (all-zero) table row
        self.nch = nch                      # AllGather chunks
        self.sch = self.npc // nch          # slots per chunk (per core)
        assert self.sch * nch == self.npc and self.sch % 128 == 0
        assert self.npad >= n


FULL_CFG = Cfg(N, 98)


def host_prep(x, edge_index, cfg):
    """Permute nodes / build per-core gather schedules on the host."""
    n = cfg.n
    src = np.asarray(edge_index[0]).astype(np.int64)
    dst = np.asarray(edge_index[1]).astype(np.int64)
    deg = np.bincount(dst, minlength=n)

    # degree-rank round-robin: rank r -> core r%8, slot r//8
    rank_order = np.argsort(deg, kind="stable")  # node ids in degree order
    r = np.arange(n)
    node_core = np.empty(n, np.int64)
    node_slot = np.empty(n, np.int64)
    # slots 0..(pads-1) of every core are left empty (pad nodes); their
    # table rows live in AllGather chunk 0 and are zeroed on device, so
    # row 0 serves as the all-zero dummy gather target.
    pads = cfg.npc - n // NCORES
    node_core[rank_order] = r % NCORES
    node_slot[rank_order] = pads + r // NCORES
    tau = node_core * cfg.npc + node_slot      # (core, slot) id of each node
    # chunk-major table row: AllGather chunk k lands at contiguous rows
    # [8*sch*k, 8*sch*(k+1))
    tau_tab = ((node_slot // cfg.sch) * (NCORES * cfg.sch)
               + node_core * cfg.sch + node_slot % cfg.sch)

    # per-edge position within its dst node's list, sorted ascending by
    # src table row so low slots only need early AllGather chunks
    order = np.lexsort((tau_tab[src], dst))
    d_sorted = dst[order]
    first = np.searchsorted(d_sorted, np.arange(n))
    k_within = np.arange(len(dst)) - first[d_sorted]
    k_e = np.empty(len(dst), np.int64)
    k_e[order] = k_within

    e_core = node_core[dst]
    e_slot = node_slot[dst]
    e_tile = e_slot // 128
    e_part = e_slot % 128

    # per-tile max degree (shared across cores so programs are identical)
    deg_cs = np.zeros((NCORES, cfg.npc), np.int64)
    deg_cs[node_core, node_slot] = deg
    d_hat = deg_cs.reshape(NCORES, cfg.tpc, 128).max(axis=(0, 2))
    d_hat = np.maximum(d_hat, 1).astype(np.int64)
    off = np.concatenate([[0], np.cumsum(d_hat)])
    nblk = int(off[-1])

    gidx = np.zeros((NCORES, 128, nblk), np.int32)   # pads -> zero row 0
    col = off[e_tile] + k_e
    gidx[e_core, e_part, col] = tau_tab[src].astype(np.int32)

    # classify gather columns by the last AllGather chunk they need, pick
    # up to EMAX "early" columns (gatherable while conv1 still runs), and
    # reorder each tile's columns early-first.
    EMAX = 512
    sch8 = NCORES * cfg.sch
    tch = cfg.sch // 128
    need_k = gidx.max(axis=(0, 1)) // sch8           # [nblk], shared
    elig = np.where(need_k <= cfg.nch - 2)[0]
    sel = set(elig[np.argsort(need_k[elig], kind="stable")][:EMAX].tolist())

    # reorder each tile's columns early-first; assign etab2 storage ids
    new_perm = []
    ne_t, e0_t, early_items = [], [], []
    e_ctr = 0
    for t in range(cfg.tpc):
        cols = list(range(off[t], off[t + 1]))
        ear = [c for c in cols if c in sel]
        late = [c for c in cols if c not in sel]
        new_perm += ear + late
        e0_t.append(e_ctr)
        ne_t.append(len(ear))
        for i, c in enumerate(ear):
            early_items.append((e_ctr, off[t] + i, int(need_k[c])))
            e_ctr += 1
    gidx = gidx[:, :, new_perm]

    # per-conv1-tile issue schedule for the early gathers
    issue = sorted(early_items, key=lambda it: (it[2], it[0]))
    sched = [[] for _ in range(cfg.tpc)]
    qi = 0
    for t1 in range(cfg.tpc):
        kav = (t1 - 3) // tch - 1
        while (qi < len(issue) and len(sched[t1]) < 8
               and issue[qi][2] <= kav):
            sched[t1].append(issue[qi])
            qi += 1
    while qi < len(issue):
        sched[cfg.tpc - 1].append(issue[qi])
        qi += 1

    # per-core x shard (feature-major, slot order) and edge src-id map
    import ml_dtypes
    xT = np.asarray(x, np.float32).T.astype(ml_dtypes.bfloat16)
    xT_own = []
    for c in range(NCORES):
        sh = np.zeros((H, cfg.npc), ml_dtypes.bfloat16)
        nodes = np.where(node_core == c)[0]
        sh[:, node_slot[nodes]] = xT[:, nodes]
        xT_own.append(sh)

    # per-core edge-expanded features for the streamed layer-1 conv:
    # xET[c][:, (col*128 + p)] = x[src of edge at (col, p)] (pads -> 0)
    sid = np.full((NCORES, 128, nblk), -1, np.int64)
    sid[e_core, e_part, col] = src
    sid = sid[:, :, new_perm]
    xT_pad = np.concatenate(
        [xT, np.zeros((H, 1), ml_dtypes.bfloat16)], axis=1)
    xET = [np.ascontiguousarray(xT_pad[:, sid[c].T.ravel()])
           for c in range(NCORES)]

    return dict(
        gidx=gidx,
        d_hat=[int(v) for v in d_hat],
        nblk=nblk,
        tau=tau,
        xT_own=xT_own,
        xET=xET,
        ne_t=ne_t,
        e0_t=e0_t,
        n_early=e_ctr,
        sched=sched,
        pads=pads,
    )


def build_program(cfg, d_hat, nblk, ne_t, e0_t, n_early, sched, pads):
    import concourse.bass as bass
    import concourse.bacc as bacc
    import concourse.mybir as mybir
    import concourse.tile as tile
    from concourse.masks import make_identity

    f32 = mybir.dt.float32
    bf16 = mybir.dt.bfloat16
    tpc, npc, nt = cfg.tpc, cfg.npc, cfg.nt

    nc = bacc.Bacc("TRN2", target_bir_lowering=False, debug=False,
                   num_devices=NCORES)

    # ---- I/O (weights/x are host-cast to bf16) ----
    xT_own = nc.dram_tensor("xT_own", [H, npc], bf16, kind="ExternalInput")
    xET = nc.dram_tensor("xET", [H, nblk * 128], bf16, kind="ExternalInput")
    gidx = nc.dram_tensor("gidx", [128, nblk], mybir.dt.int32,
                          kind="ExternalInput")
    wnames = {}
    for l in (1, 2):
        for w in ("Wqv", "Wk", "Ws", "Wl"):
            shape = [H, 128] if w == "Wqv" else [H, H]
            wnames[f"{w}{l}"] = nc.dram_tensor(f"{w}{l}", shape, bf16,
                                               kind="ExternalInput")
        wnames[f"b{l}r"] = nc.dram_tensor(f"b{l}r", [1, H], bf16,
                                          kind="ExternalInput")
        wnames[f"bl{l}c"] = nc.dram_tensor(f"bl{l}c", [H, 1], f32,
                                           kind="ExternalInput")
    out_shard = nc.dram_tensor("out_shard", [npc, H], f32,
                               kind="ExternalOutput")

    # ---- internal DRAM (layer-2 q|v table only; layer 1 is streamed) ----
    qv2_t = nc.dram_tensor("qv2_t", [nt, 128], bf16, addr_space="Shared")
    qv2_sh = nc.dram_tensor("qv2_sh", [npc, 128], bf16)
    etab2 = nc.dram_tensor("etab2", [max(1, n_early) * 128, 128], bf16)

    with tile.TileContext(nc) as tc:
        cp = tc.alloc_tile_pool(name="const", bufs=1)

        identity = cp.tile([128, 128], f32)
        make_identity(nc, identity[:])
        identity64 = cp.tile([64, 64], f32)
        make_identity(nc, identity64[:])
        ones_row = cp.tile([1, 128], bf16)
        nc.vector.memset(ones_row[:], 1.0)
        zeros128 = cp.tile([128, 128], bf16)
        nc.vector.memset(zeros128[:], 0.0)

        gidx_sb = cp.tile([128, nblk], mybir.dt.int32)
        nc.sync.dma_start(out=gidx_sb[:], in_=gidx.ap()[:])

        # x shard and h1 live in SBUF for the whole program
        xsb = cp.tile([H, npc], bf16)
        nc.sync.dma_start(out=xsb[:], in_=xT_own.ap()[:])
        h1_all = cp.tile([H, npc], bf16)

        wt = {}
        for l in (1, 2):
            for w in ("Wqv", "Wk", "Ws", "Wl"):
                shape = [H, 128] if w == "Wqv" else [H, H]
                wt[f"{w}{l}"] = cp.tile(shape, bf16, name=f"{w}{l}",
                                        tag=f"{w}{l}")
                nc.sync.dma_start(out=wt[f"{w}{l}"][:],
                                  in_=wnames[f"{w}{l}"].ap()[:])
            wt[f"b{l}r"] = cp.tile([1, H], bf16, name=f"b{l}r", tag=f"b{l}r")
            nc.sync.dma_start(out=wt[f"b{l}r"][:], in_=wnames[f"b{l}r"].ap()[:])
            wt[f"bl{l}c"] = cp.tile([H, 1], f32, name=f"bl{l}c", tag=f"bl{l}c")
            nc.sync.dma_start(out=wt[f"bl{l}c"][:],
                              in_=wnames[f"bl{l}c"].ap()[:])

        def table_exchange_chunk(k):
            """AllGather shard chunk k into contiguous table rows."""
            r0, r1 = k * cfg.sch, (k + 1) * cfg.sch
            nc.gpsimd.collective_compute(
                "AllGather",
                mybir.AluOpType.bypass,
                replica_groups=[list(range(NCORES))],
                ins=[qv2_sh.ap()[r0:r1, :]],
                outs=[qv2_t.ap()[NCORES * r0:NCORES * r1, :]],
            )

        # -------- layer 1: streamed conv (no gather; token-major) ----------
        def conv1_streamed():
            G = 8   # qv chunks per PSUM group
            TCH = cfg.sch // 128  # tiles per AllGather chunk
            wqv, wk, ws, wl = (wt["Wqv1"], wt["Wk1"], wt["Ws1"], wt["Wl1"])
            brow, blc = wt["b1r"], wt["bl1c"]
            with tc.tile_pool(name="p1e", bufs=2) as pe_, \
                 tc.tile_pool(name="p1s", bufs=3) as pbs, \
                 tc.tile_pool(name="p1p", bufs=2, space="PSUM") as pbp:
                off_c = 0
                for t in range(tpc):
                    dh = d_hat[t]
                    hot = xsb[:, t * 128:(t + 1) * 128]

                    # per-edge x features of this tile, streamed in
                    xe = pe_.tile([H, dh * 128], bf16, tag="xe", bufs=6)
                    nc.sync.dma_start(
                        out=xe[:],
                        in_=xET.ap()[:, off_c * 128:(off_c + dh) * 128])

                    # k tile for this dst tile: [128 n, 64 h]
                    kps = pbp.tile([128, H], f32, tag="kps", space="PSUM",
                                   bufs=1)
                    nc.tensor.matmul(kps[:], lhsT=hot, rhs=wk[:],
                                     start=True, stop=True)
                    ksb = pbs.tile([128, H], bf16, tag="ksb", bufs=2)
                    nc.scalar.activation(ksb[:], kps[:],
                                         mybir.ActivationFunctionType.Copy)

                    msg = pbs.tile([128, dh, H], bf16, tag="msg", bufs=3)
                    for j0 in range(0, dh, G):
                        gc = min(G, dh - j0)
                        # expand gc chunks of 128 edges: [128 e, 128 qv] each
                        psg = pbp.tile([128, G * 128], f32, tag="ps",
                                       space="PSUM", bufs=2)
                        for i in range(gc):
                            nc.tensor.matmul(
                                psg[:, i * 128:(i + 1) * 128],
                                lhsT=xe[:, (j0 + i) * 128:(j0 + i + 1) * 128],
                                rhs=wqv[:], start=True, stop=True)
                        psv = psg[:, 0:gc * 128].rearrange(
                            "p (g e) -> p g e", e=128)
                        # sigarg = q + k[dst]; sigmoid; msg = sig * v
                        sa = pbs.tile([128, G, H], bf16, tag="sa", bufs=3)
                        kb = ksb[:].rearrange("p (o h) -> p o h", o=1)
                        kb = bass.AP(kb.tensor, kb.offset,
                                     [kb.ap[0], [0, gc], kb.ap[2]])
                        nc.vector.tensor_tensor(
                            out=sa[:, 0:gc, :], in0=psv[:, :, 0:H], in1=kb,
                            op=mybir.AluOpType.add)
                        sg = pbs.tile([128, G, H], bf16, tag="sg", bufs=3)
                        nc.scalar.activation(
                            sg[:, 0:gc, :], sa[:, 0:gc, :],
                            mybir.ActivationFunctionType.Sigmoid)
                        nc.vector.tensor_tensor(
                            out=msg[:, j0:j0 + gc, :], in0=sg[:, 0:gc, :],
                            in1=psv[:, :, H:128],
                            op=mybir.AluOpType.mult)

                    # agg[p, h] = sum_j msg[p, j, h]
                    agg = pbs.tile([128, H], f32, tag="agg")
                    mt = msg[:].rearrange("p k h -> p h k")
                    nc.vector.tensor_reduce(
                        out=agg[:], in_=mt, axis=mybir.AxisListType.X,
                        op=mybir.AluOpType.add)

                    # conv out = agg + x@Ws + b  (token-major [128 n, 64 h])
                    cps = pbp.tile([128, H], f32, tag="cps", space="PSUM",
                                   bufs=1)
                    nc.tensor.matmul(cps[:], lhsT=hot, rhs=ws[:],
                                     start=True, stop=False)
                    nc.tensor.matmul(cps[:], lhsT=ones_row[:], rhs=brow[:],
                                     start=False, stop=True)
                    hc = pbs.tile([128, H], f32, tag="hc")
                    nc.vector.tensor_tensor(out=hc[:], in0=agg[:], in1=cps[:],
                                            op=mybir.AluOpType.add)

                    # transpose to feature-major, then linear + relu
                    tps = pbp.tile([H, 128], f32, tag="tps", space="PSUM",
                                   bufs=1)
                    nc.tensor.transpose(out=tps[:], in_=hc[:],
                                        identity=identity[:])
                    hcT = pbs.tile([H, 128], bf16, tag="hcT")
                    nc.scalar.activation(hcT[:], tps[:],
                                         mybir.ActivationFunctionType.Copy)
                    lps = pbp.tile([H, 128], f32, tag="lps", space="PSUM",
                                   bufs=1)
                    nc.tensor.matmul(lps[:], lhsT=wl[:], rhs=hcT[:],
                                     start=True, stop=True)

                    # h1 tile -> resident SBUF (feature-major, bf16)
                    h1t = h1_all[:, t * 128:(t + 1) * 128]
                    nc.scalar.activation(
                        h1t, lps[:],
                        mybir.ActivationFunctionType.Relu,
                        bias=blc[:])
                    # fused layer-2 table shard build (token-major rows)
                    qps = pbp.tile([128, 128], f32, tag="ps",
                                   space="PSUM", bufs=2)
                    nc.tensor.matmul(qps[:], lhsT=h1t, rhs=wt["Wqv2"][:],
                                     start=True, stop=True)
                    st2 = pbs.tile([128, 128], bf16, tag="st2")
                    nc.scalar.activation(st2[:], qps[:],
                                         mybir.ActivationFunctionType.Copy)
                    nc.sync.dma_start(
                        out=qv2_sh.ap()[t * 128:(t + 1) * 128, :],
                        in_=st2[:])
                    if t == 0:
                        # zero the pad-node table rows (incl. dummy row 0)
                        nc.sync.dma_start(out=qv2_sh.ap()[0:pads, :],
                                          in_=zeros128[0:pads, :])
                    if (t + 1) % TCH == 0:
                        table_exchange_chunk((t + 1) // TCH - 1)
                    # early layer-2 gathers: Q7 is otherwise idle in conv1
                    for (e, colid, needk) in sched[t]:
                        qe = pe_.tile([128, 128], bf16, tag="qe", bufs=8)
                        nc.gpsimd.indirect_dma_start(
                            out=qe[:], out_offset=None,
                            in_=qv2_t.ap()[
                                0:(needk + 1) * NCORES * cfg.sch, :],
                            in_offset=bass.IndirectOffsetOnAxis(
                                ap=gidx_sb[:, colid:colid + 1], axis=0),
                        )
                        nc.sync.dma_start(
                            out=etab2.ap()[e * 128:(e + 1) * 128, :],
                            in_=qe[:])
                    off_c += dh

        # -------- layer 2: gather-based conv (token-major) -----------------
        def conv_layer2():
            wk, ws, wl = wt["Wk2"], wt["Ws2"], wt["Wl2"]
            brow, blc = wt["b2r"], wt["bl2c"]
            with tc.tile_pool(name="pb2", bufs=2) as pb, \
                 tc.tile_pool(name="pbs2", bufs=3) as pbs, \
                 tc.tile_pool(name="pbp2", bufs=2, space="PSUM") as pbp:
                off_c = 0
                for t in range(tpc):
                    dh = d_hat[t]
                    hot = h1_all[:, t * 128:(t + 1) * 128]

                    # k tile for this dst tile: [128 n, 64 h]
                    kps = pbp.tile([128, H], f32, tag="kps", space="PSUM")
                    nc.tensor.matmul(kps[:], lhsT=hot, rhs=wk[:],
                                     start=True, stop=True)
                    ksb = pbs.tile([128, H], bf16, tag="ksb", bufs=4)
                    nc.scalar.activation(ksb[:], kps[:],
                                         mybir.ActivationFunctionType.Copy)

                    # q|v rows of this tile's edges: early columns were
                    # pre-gathered into etab2 during conv1; gather the rest
                    qvg_f = pb.tile([128, dh * 128], bf16, tag="qvg", bufs=8)
                    ne = ne_t[t]
                    if ne:
                        e0 = e0_t[t]
                        nc.sync.dma_start(
                            out=qvg_f[:, 0:ne * 128].rearrange(
                                "p (c e) -> p c e", e=128),
                            in_=etab2.ap()[e0 * 128:(e0 + ne) * 128,
                                           :].rearrange(
                                "(c p) e -> p c e", p=128))
                    for j in range(ne, dh):
                        nc.gpsimd.indirect_dma_start(
                            out=qvg_f[:, j * 128:(j + 1) * 128],
                            out_offset=None,
                            in_=qv2_t.ap()[:, :],
                            in_offset=bass.IndirectOffsetOnAxis(
                                ap=gidx_sb[:, off_c + j:off_c + j + 1],
                                axis=0),
                        )
                    qvg = qvg_f[:].rearrange("p (a b) -> p a b", b=128)

                    # sigarg = k[dst] + q ;  sig = sigmoid(sigarg)
                    sigarg = pbs.tile([128, dh, H], bf16, tag="sigarg", bufs=4)
                    kb = ksb[:].rearrange("p (o h) -> p o h", o=1)
                    kb = bass.AP(kb.tensor, kb.offset,
                                 [kb.ap[0], [0, dh], kb.ap[2]])
                    nc.vector.tensor_tensor(
                        out=sigarg[:], in0=qvg[:, :, 0:H], in1=kb,
                        op=mybir.AluOpType.add)
                    sig = pbs.tile([128, dh, H], bf16, tag="sig", bufs=4)
                    nc.scalar.activation(
                        sig[:], sigarg[:],
                        mybir.ActivationFunctionType.Sigmoid)
                    # msg = sig * v
                    msg = pbs.tile([128, dh, H], bf16, tag="msg", bufs=4)
                    nc.vector.tensor_tensor(
                        out=msg[:], in0=sig[:], in1=qvg[:, :, H:128],
                        op=mybir.AluOpType.mult)
                    # agg[p, h] = sum_k msg[p, k, h]
                    agg = pbs.tile([128, H], f32, tag="agg")
                    mt = msg[:].rearrange("p k h -> p h k")
                    nc.vector.tensor_reduce(
                        out=agg[:], in_=mt, axis=mybir.AxisListType.X,
                        op=mybir.AluOpType.add)

                    # conv out = agg + h1@Ws + b  (token-major [128 n, 64 h])
                    cps = pbp.tile([128, H], f32, tag="cps", space="PSUM")
                    nc.tensor.matmul(cps[:], lhsT=hot, rhs=ws[:],
                                     start=True, stop=False)
                    nc.tensor.matmul(cps[:], lhsT=ones_row[:], rhs=brow[:],
                                     start=False, stop=True)
                    hc = pbs.tile([128, H], f32, tag="hc")
                    nc.vector.tensor_tensor(out=hc[:], in0=agg[:], in1=cps[:],
                                            op=mybir.AluOpType.add)

                    # transpose to feature-major, then linear + relu
                    tps = pbp.tile([H, 128], f32, tag="tps", space="PSUM",
                                   bufs=1)
                    nc.tensor.transpose(out=tps[:], in_=hc[:],
                                        identity=identity[:])
                    hcT = pbs.tile([H, 128], bf16, tag="hcT")
                    nc.scalar.activation(hcT[:], tps[:],
                                         mybir.ActivationFunctionType.Copy)
                    lps = pbp.tile([H, 128], f32, tag="lps", space="PSUM")
                    nc.tensor.matmul(lps[:], lhsT=wl[:], rhs=hcT[:],
                                     start=True, stop=True)

                    h2T = pbs.tile([H, 128], f32, tag="h2T")
                    nc.scalar.activation(
                        h2T[:], lps[:],
                        mybir.ActivationFunctionType.Relu,
                        bias=blc[:])
                    ops = pbp.tile([128, H], f32, tag="kps", space="PSUM")
                    nc.tensor.transpose(out=ops[:], in_=h2T[:],
                                        identity=identity64[:])
                    osb = pbs.tile([128, H], f32, tag="osb")
                    nc.scalar.activation(osb[:], ops[:],
                                         mybir.ActivationFunctionType.Copy)
                    nc.sync.dma_start(
                        out=out_shard.ap()[t * 128:(t + 1) * 128, :],
                        in_=osb[:])
                    off_c += dh

        conv1_streamed()
        nc.sync.dma_start(out=qv2_t.ap()[cfg.npad:nt, :], in_=zeros128[:])
        tc.strict_bb_all_engine_barrier()
        conv_layer2()

        cp.release()

    nc.compile()
    return nc


def _pack_inputs(prep, inputs, cfg):
    """Build the 8 per-core input maps."""
    import ml_dtypes
    bf16 = ml_dtypes.bfloat16
    base = {}
    for l, (wq, wv, wk, ws, b, wl, bl) in {
        1: ("Wq1", "Wv1", "Wk1", "Ws1", "b1", "Wl1", "bl1"),
        2: ("Wq2", "Wv2", "Wk2", "Ws2", "b2", "Wl2", "bl2"),
    }.items():
        base[f"Wqv{l}"] = np.ascontiguousarray(
            np.concatenate([np.asarray(inputs[wq], np.float32),
                            np.asarray(inputs[wv], np.float32)],
                           axis=1)).astype(bf16)
        base[f"Wk{l}"] = np.asarray(inputs[wk], np.float32).astype(bf16)
        base[f"Ws{l}"] = np.asarray(inputs[ws], np.float32).astype(bf16)
        base[f"Wl{l}"] = np.asarray(inputs[wl], np.float32).astype(bf16)
        base[f"b{l}r"] = np.asarray(inputs[b], np.float32).reshape(1, H).astype(bf16)
        base[f"bl{l}c"] = np.ascontiguousarray(
            np.asarray(inputs[bl], np.float32).reshape(H, 1))

    in_maps = []
    for c in range(NCORES):
        m = dict(base)
        m["xT_own"] = prep["xT_own"][c]
        m["xET"] = prep["xET"][c]
        m["gidx"] = np.ascontiguousarray(prep["gidx"][c])
        in_maps.append(m)
    return in_maps


def run(inputs, cfg=FULL_CFG, sim=False, trace=False):
    from concourse import bass_utils

    x = np.asarray(inputs["x"], np.float32)
    prep = host_prep(x, inputs["edge_index"], cfg)
    nc = build_program(cfg, prep["d_hat"], prep["nblk"], prep["ne_t"],
                       prep["e0_t"], prep["n_early"], prep["sched"],
                       prep["pads"])
    in_maps = _pack_inputs(prep, inputs, cfg)

    if sim:
        from concourse.bass_interp import MultiCoreSim
        ms = MultiCoreSim(nc, num_cores=NCORES, trace=False)
        for c in range(NCORES):
            for name, arr in in_maps[c].items():
                ms.cores[c].tensor(name)[:] = arr
        ms.simulate(check_with_hw=False)
        shards = [np.array(ms.cores[c].tensor("out_shard")) for c in
                  range(NCORES)]
        res = None
    else:
        if trace:
            try:
                sys.path.insert(0, "/root/problem")
                import ntff_hook  # noqa: F401
            except Exception:
                trace = False
        res = bass_utils.run_bass_kernel_spmd(
            nc, in_maps, core_ids=list(range(NCORES)), trace=trace)
        shards = [res.results[c]["out_shard"] for c in range(NCORES)]

    full = np.concatenate(shards, axis=0)     # [npad, H] in permuted order
    out = np.empty((cfg.n, H), np.float32)
    out[:, :] = full[prep["tau"], :]
    return out, res


def kernel(**inputs):
    out, _ = run(inputs, FULL_CFG, sim=False, trace=False)
    return out.astype(np.float32)
